# revision 63
# baseline (speedup 1.0000x reference)
"""Trainium2 Bass kernel for the JointLoss problem (contrastive NT-Xent + 2 MSE terms).

kernel(representation, xrecon, xorig) -> (loss, closs, recon_loss, zrecon_loss)

Strategy (8 NeuronCores, SPMD — one NEFF, per-core variation only via inputs):
  - Host normalizes the representations (the sharding hint's "all-gather of the
    normalized representations" — normalization happens before the gather in the
    data-parallel formulation), scales by S=16 so fp8 e4m3 stays in its normal
    range, and ships z^T slabs per core.  sim/tau is recovered by folding
    1/(S^2 tau) into the exp's constant scale.
  - Each core computes a (512, 2560) slab of q = (S z_i)·(S z_j) against column
    chunks [partner, own, +1, +2, +3] using fp8 DoubleRow matmuls (K=256 per
    instruction).  exp runs on Act with per-call row-sum accumulators; chunks
    +1..+3 write fp8 exp tiles whose column sums (one DoubleRow ones-matmul
    per rr pair) supply the transposed contributions to other cores' rows.
  - Column sums run as N=1 DoubleRow transpose-reduce matmuls (exp tile as
    weights, ones moving) into partition-major PSUM columns; one single-shot
    accumulation group per bank (a start marks the whole bank's zero region).
  - Positives come from the diagonal of the partner block, extracted from PSUM
    with an identity mask multiply + free-axis reduce on DVE.  (The native
    tensor_tensor_reduce ISA op crashes the exec unit on this platform.)
  - MSE terms: bf16 subtract + square on DVE over interleaved xr|xo quarters
    as they land; sums via accumulating ones-matmul column sums on PE (dx)
    and a per-partition DVE reduce (dz); host adds the partials.
  - Inputs are packed into 3 DRAM tensors and streamed as few large DMAs (SP
    issues cost ~650ns each, serial): the own chunk lands first and feeds a
    dedicated PSUM bank so the first exp starts at ~4us; exp runs gapless on
    Act (the critical engine) until ~18us.
  - All per-core partials leave through two small DMAs (accA via the Act DGE
    queue, accD via SP); host combine sums the 8 cores' row sums + pushed
    column sums, subtracts exp(1/tau) for the self column, takes log, and
    adds the two MSE scalars.
"""

import math

import ml_dtypes
import numpy as np

TAU = 0.5
EPS = 1e-8
N = 2048
TWO_N = 4096
D = 512
NCORES = 8
CH = 512
S = 16.0  # fp8 pre-scale for normalized vectors
QS = 1.0 / (S * S * TAU)  # exp input scale

_CACHE = {}


def _build_nc():
    import concourse.bacc as bacc
    import concourse.mybir as mybir
    import concourse.tile as tile

    F32 = mybir.dt.float32
    BF16 = mybir.dt.bfloat16
    FP8 = mybir.dt.float8e4
    OP = mybir.AluOpType
    AF = mybir.ActivationFunctionType
    AX = mybir.AxisListType
    DR = mybir.MatmulPerfMode.DoubleRow

    nc = bacc.Bacc("TRN2", target_bir_lowering=False, debug=False)
    # ztp per-partition layout (fp8 bytes): 5 chunks of [d(4) x 512 cols],
    # stream order [own, partner, +1, +2, +3] — the own chunk (weights + the
    # first exp slab) lands first.  ztp[p] holds z^T rows {d*128+p}.
    ztp = nc.dram_tensor("ztp", [128, 10240], FP8, kind="ExternalInput")
    # X: [q(4) x io(2) x 1024] bf16 per partition — xrecon/xorig quarters
    xin = nc.dram_tensor("xin", [128, 8192], BF16, kind="ExternalInput")
    # Z: [io(2) x 1024] bf16 — zi | zj
    zin = nc.dram_tensor("zin", [128, 2048], BF16, kind="ExternalInput")
    # 128x128 identity from the host (gpsimd affine_select would drag the
    # Pool engine into the final drain barrier)
    idf = nc.dram_tensor("idf", [128, 128], F32, kind="ExternalInput")
    out1a = nc.dram_tensor("out1a", [128, 16], F32, kind="ExternalOutput")
    out1b = nc.dram_tensor("out1b", [128, 18], F32, kind="ExternalOutput")

    with tile.TileContext(nc) as tc:
        with (
            tc.tile_pool(name="singles", bufs=1) as singles,
            tc.tile_pool(name="msep", bufs=2) as msep,
            tc.tile_pool(name="extp", bufs=2) as extp,
            tc.tile_pool(name="mpsum", bufs=2, space="PSUM") as mpsum,
            tc.tile_pool(name="cpsum", bufs=1, space="PSUM") as cpsum,
            tc.tile_pool(name="qpsum", bufs=1, space="PSUM") as qpsum,
            tc.tile_pool(name="qxpsum", bufs=1, space="PSUM") as qxpsum,
            tc.tile_pool(name="apsum", bufs=1, space="PSUM") as apsum,
        ):
            identf = singles.tile([128, 128], F32, tag="identf")
            # dual-fp8 ldweights requires the k-pair stride in the weights AP
            # to be 16-byte aligned (walrus s3_lw_dual_fp8_restrictions), so
            # the ones live in a [128, 2, 16] tile sliced to [:, :, 0:2]
            ones8 = singles.tile([128, 2, 16], FP8, tag="ones8")
            nc.vector.memset(ones8, 1.0)
            ones_bf = singles.tile([128, 1], BF16, tag="ones_bf")
            nc.vector.memset(ones_bf, 1.0)
            # accA: 0-11 eacc[rr*3+blk] (Act accumulators); accD: 0-3 pos[rr],
            # 4 dz partials (DVE) — separate tiles so the tile-granular dep
            # tracker doesn't serialize Act and DVE against each other
            accA = singles.tile([128, 16], F32, tag="accA")
            # accD: 0-3 pos[rr], 4 dz partials, 5-16 exp column-sum blocks,
            # 17 dx partials
            accD = singles.tile([128, 18], F32, tag="accD")
            ebJ = []
            for j in range(2):
                t = singles.tile([128, 2, 1536], FP8, tag=f"ebJ_{j}")
                ebJ.append(t)

            ebA_t = []
            for i in range(2):
                t = singles.tile([128, 1024], FP8, tag=f"ebA_{i}")
                ebA_t.append(t)

            # explicit zero bias AP for Exp: the float-bias path materializes
            # a const tensor whose DMA lands at the head of the input stream,
            # delaying every zt transfer by one slot
            zbias = singles.tile([128, 1], F32, tag="zbias")
            nc.vector.memset(zbias, 0.0)

            # act-table warmup: a no-dep Exp at t~0 so LoadActFuncSet isn't
            # gated behind the first PSUM tile
            warm = singles.tile([128, 1], F32, tag="warm")
            nc.vector.memset(warm, 0.0)
            nc.scalar.activation(warm, warm, AF.Exp, bias=zbias)

            # --- input DMA stream (all on the SP hardware DGE) ---
            zt_t = {}
            for ch in range(5):
                t = singles.tile([128, 4, 512], FP8, tag=f"zt_{ch}")
                zt_t[ch] = t
            for k, ch in enumerate((1, 0, 2, 3, 4)):
                nc.sync.dma_start(zt_t[ch], ztp[:, 2048 * k : 2048 * (k + 1)])
            nc.sync.dma_start(identf, idf[:, :])
            zq = singles.tile([128, 2, 1024], BF16, tag="zq")
            nc.sync.dma_start(zq, zin[:, :])
            xq = []
            for k in range(4):
                t = singles.tile([128, 2, 1024], BF16, tag=f"xq_{k}")
                xq.append(t)
                nc.sync.dma_start(t, xin[:, 2048 * k : 2048 * (k + 1)])

            psA1_0 = apsum.tile([128, 512], F32, tag="psA1_0")
            # one [128,24] tile: cols 0-11 = j0 wave, 12-23 = j1 wave (both
            # single-shot groups, so they can share a bank)
            qq = qpsum.tile([128, 24], F32, tag="qq")
            qdx = qxpsum.tile([128, 1], F32, tag="qdx")
            dxq_t = []
            for k in range(4):
                t = singles.tile([128, 1024], BF16, tag=f"dxq_{k}")
                dxq_t.append(t)
            dzs = singles.tile([128, 1024], BF16, tag="dzs")

            def colsum_wave(j):
                # column sums of chunks +1/+2/+3 exp tiles over one rr pair,
                # partition-major: transpose-reduce (exp tile as weights, ones
                # moving) gives [128,1] per 128-column block
                # one accumulation group per PSUM bank (start marks the whole
                # bank's zero region): each wave owns a bank, summed on DVE
                for ch in range(3):
                    for blk in range(4):
                        nc.tensor.matmul(
                            qq[:, 12 * j + 4 * ch + blk : 12 * j + 4 * ch + blk + 1],
                            ebJ[j][:, :, CH * ch + 128 * blk : CH * ch + 128 * (blk + 1)],
                            ones8[:, :, 0:1],
                            start=True,
                            stop=True,
                            perf_mode=DR,
                        )

            def mm_block(rr, chunks, dst_of):
                for dd in range(2):
                    w = zt_t[1][:, 2 * dd : 2 * dd + 2, 128 * rr : 128 * (rr + 1)]
                    for ch in chunks:
                        nc.tensor.matmul(
                            dst_of(ch),
                            w,
                            zt_t[ch][:, 2 * dd : 2 * dd + 2, :],
                            start=(dd == 0),
                            stop=(dd == 1),
                            perf_mode=DR,
                        )

            def exp_to(dst, src, col):
                nc.scalar.activation(
                    dst,
                    src,
                    AF.Exp,
                    bias=zbias,
                    scale=QS,
                    accum_out=accA[:, col : col + 1],
                )

            def pos_extract(rr, psA):
                # positives: diagonal of the partner block (raw q, pre-exp)
                ext = extp.tile([128, 128], F32, tag="ext")
                nc.vector.tensor_tensor(
                    ext, psA[:, 128 * rr : 128 * (rr + 1)], identf, OP.mult
                )
                nc.vector.reduce_sum(accD[:, rr : rr + 1], ext, axis=AX.X)

            def mse_sub_sq(src, sq):
                d = msep.tile([128, 1024], BF16, tag="d")
                nc.vector.tensor_tensor(d, src[:, 0], src[:, 1], OP.subtract)
                nc.vector.tensor_tensor(sq, d, d, OP.mult)

            def qcap_mm(k):
                # per-128-col-block transpose-reduce of dx^2, all blocks
                # accumulated into one [128,1] column
                for blk in range(8):
                    nc.tensor.matmul(
                        qdx,
                        dxq_t[k][:, 128 * blk : 128 * (blk + 1)],
                        ones_bf,
                        start=(k == 0 and blk == 0),
                        stop=(k == 3 and blk == 7),
                    )

            for rr in range(4):
                psA = mpsum.tile([128, 1024], F32, tag="ps")
                psB = mpsum.tile([128, 1024], F32, tag="ps")
                psC = cpsum.tile([128, 512], F32, tag="psC")
                dstA = lambda ch: psA[:, CH * ch : CH * (ch + 1)]
                dstBC = lambda ch: psB[:, CH * (ch - 2) : CH * (ch - 1)] if ch < 4 else psC

                ebA = ebA_t[rr % 2]
                if rr == 0:
                    # own chunk into a dedicated bank: its transfer lands first
                    # and feeds both the weights and the first exp slab
                    mm_block(rr, (1,), lambda ch: psA1_0)
                    exp_to(ebA[:, 512:1024], psA1_0, 0)
                    mm_block(rr, (0,), lambda ch: psA[:, 0:512])
                    # chunk 4 ahead of 2/3: the B exp is gated on the zt3 DMA
                    # chain either way, C's bank (psC=b1) is free immediately
                    mm_block(rr, (4,), dstBC)
                    mm_block(rr, (2, 3), dstBC)
                elif rr < 3:
                    mm_block(rr, (0, 1), dstA)
                    mm_block(rr, (2, 3, 4), dstBC)
                else:
                    # rr3: colsum-feeding blocks B/C first so the last colsum
                    # wave and PSUM->SBUF copies overlap with exp A(3)
                    mm_block(rr, (2, 3, 4), dstBC)
                    # fill the PE wait for the psum ring with ready dx sums
                    qcap_mm(0)
                    qcap_mm(1)
                    mm_block(rr, (0, 1), dstA)

                # interleave MSE work into the DVE queue by DMA readiness
                if rr == 1:
                    mse_sub_sq(zq, dzs)
                    nc.vector.reduce_sum(accD[:, 4:5], dzs, axis=AX.X)
                    mse_sub_sq(xq[0], dxq_t[0])
                elif rr == 2:
                    mse_sub_sq(xq[1], dxq_t[1])
                    mse_sub_sq(xq[2], dxq_t[2])
                elif rr == 3:
                    mse_sub_sq(xq[3], dxq_t[3])

                ebr = ebJ[rr // 2][:, rr % 2]
                if rr == 0:
                    exp_to(ebA[:, 0:512], psA[:, 0:512], 1)
                    exp_to(ebr[:, 1024:1536], psC, 3)
                    exp_to(ebr[:, 0:1024], psB, 2)
                    pos_extract(rr, psA)
                elif rr < 3:
                    exp_to(ebA, psA, 1 + 3 * rr)
                    exp_to(ebr[:, 0:1024], psB, 2 + 3 * rr)
                    exp_to(ebr[:, 1024:1536], psC, 3 + 3 * rr)
                    # pos AFTER the exps: PSUM readers are serialized in
                    # emission order, a DVE reader first would gate exp A
                    pos_extract(rr, psA)
                else:
                    exp_to(ebr[:, 0:1024], psB, 2 + 3 * rr)
                    exp_to(ebr[:, 1024:1536], psC, 3 + 3 * rr)
                    colsum_wave(1)
                    exp_to(ebA, psA, 1 + 3 * rr)
                    pos_extract(rr, psA)
                if rr == 1:
                    colsum_wave(0)

            qcap_mm(2)
            qcap_mm(3)
            nc.vector.tensor_copy(accD[:, 5:17], qq[:, 0:12])
            nc.vector.tensor_tensor(accD[:, 5:17], accD[:, 5:17], qq[:, 12:24], OP.add)
            nc.vector.tensor_copy(accD[:, 17:18], qdx)

            nc.sync.dma_start(out1b[:, :], accD)
            nc.scalar.dma_start(out1a[:, :], accA)

    # Force a single activation-function table: Exp and Copy both live in the
    # natural_log_exp_and_others set, but the load-insertion pass greedily
    # picks the first set per function and would reload between them.
    import concourse.bacc as bacc_mod
    from concourse.hw_specs import get_activation_tables

    real = get_activation_tables(nc.m.arch)
    target = "natural_log_exp_and_others"
    assert target in real
    filtered = {k: (v if k == target else set()) for k, v in real.items()}
    orig = bacc_mod.get_activation_tables
    bacc_mod.get_activation_tables = lambda arch: filtered
    try:
        nc.compile()
    finally:
        bacc_mod.get_activation_tables = orig
    return nc


def _get_nc():
    if "nc" not in _CACHE:
        _CACHE["nc"] = _build_nc()
    return _CACHE["nc"]


def make_in_maps(representation, xrecon, xorig):
    rep = np.ascontiguousarray(np.asarray(representation, dtype=np.float32))
    nrm = np.maximum(np.linalg.norm(rep, axis=1, keepdims=True), EPS)
    u = (rep / nrm) * S
    uq = u.astype(ml_dtypes.float8_e4m3)
    UT = np.ascontiguousarray(uq.T)  # (512, 4096) fp8
    xrec = np.asarray(xrecon, dtype=np.float32).astype(ml_dtypes.bfloat16)
    xorg = np.asarray(xorig, dtype=np.float32).astype(ml_dtypes.bfloat16)
    repb = rep.astype(ml_dtypes.bfloat16)
    in_maps = []
    for c in range(NCORES):
        partner = (c + 4) % 8
        order = [partner, c, (c + 1) % 8, (c + 2) % 8, (c + 3) % 8]
        ut_c = np.concatenate([UT[:, CH * p : CH * (p + 1)] for p in order], axis=1)
        # (512, 2560) -> [ch, p, d, col]: zt[ch][p][d*512+c]
        zt_c = ut_c.reshape(4, 128, 5, 512).transpose(2, 1, 0, 3)  # [ch, p, d, c]
        # ztp stream order: own, partner, +1, +2, +3
        ztp = np.concatenate(
            [zt_c[ch].reshape(128, 2048) for ch in (1, 0, 2, 3, 4)], axis=1
        )
        # xr/xo rows 512c..512c+511 packed [p, rr, 1024] then quartered along
        # the flattened free dim with io interleaved per quarter
        xr_c = xrec[CH * c : CH * (c + 1)].reshape(4, 128, 1024).transpose(1, 0, 2).reshape(128, 4096)
        xo_c = xorg[CH * c : CH * (c + 1)].reshape(4, 128, 1024).transpose(1, 0, 2).reshape(128, 4096)
        xin = np.stack(
            [xr_c.reshape(128, 4, 1024), xo_c.reshape(128, 4, 1024)], axis=2
        ).reshape(128, 8192)
        zi_c = repb[256 * c : 256 * (c + 1)].reshape(2, 128, D).transpose(1, 0, 2).reshape(128, 1024)
        zj_c = repb[2048 + 256 * c : 2048 + 256 * (c + 1)].reshape(2, 128, D).transpose(1, 0, 2).reshape(128, 1024)
        zin = np.concatenate([zi_c, zj_c], axis=1)
        in_maps.append(
            {
                "ztp": np.ascontiguousarray(ztp),
                "xin": np.ascontiguousarray(xin),
                "zin": np.ascontiguousarray(zin),
                "idf": np.eye(128, dtype=np.float32),
            }
        )
    return in_maps


def combine_outputs(results):
    """results: list of 8 dicts with out1a/out1b/ocol partials."""
    E2 = math.exp(1.0 / TAU)
    denom = np.zeros(TWO_N, dtype=np.float64)
    pos = np.zeros(TWO_N, dtype=np.float64)
    dxs = 0.0
    dzs = 0.0
    for c in range(NCORES):
        a = np.asarray(results[c]["out1a"], dtype=np.float64)  # [128, 16]
        b = np.asarray(results[c]["out1b"], dtype=np.float64)  # [128, 18]
        # partition p, row group rr -> global row 512c + 128rr + p
        # accA columns: rr0 -> 0..3, rr k -> {1+3k, 2+3k, 3+3k}
        rsum = np.stack(
            [a[:, 0:4].sum(axis=1)]
            + [a[:, 1 + 3 * k : 4 + 3 * k].sum(axis=1) for k in (1, 2, 3)],
            axis=1,
        )  # [p, rr]
        denom[CH * c : CH * (c + 1)] += rsum.T.reshape(-1)
        pos[CH * c : CH * (c + 1)] = b[:, 0:4].T.reshape(-1)
        # colsum block (ch, blk) -> global columns of chunk c+1+ch
        for ch in range(3):
            m = (c + 1 + ch) % NCORES
            cs = b[:, 5 + 4 * ch : 5 + 4 * (ch + 1)]  # [128 m, 4 blk]
            denom[CH * m : CH * (m + 1)] += cs.T.reshape(-1)
        dxs += b[:, 17].sum()
        dzs += b[:, 4].sum()
    denom -= E2
    closs = (np.log(denom) - pos * QS).sum() / TWO_N
    recon = dxs / TWO_N
    zrec = dzs / N
    loss = recon + closs + zrec
    f = np.float32
    return (f(loss), f(closs), f(recon), f(zrec))


def kernel(representation, xrecon, xorig):
    from concourse.bass_utils import run_bass_kernel_spmd

    nc = _get_nc()
    in_maps = make_in_maps(representation, xrecon, xorig)
    res = run_bass_kernel_spmd(nc, in_maps, core_ids=list(range(NCORES)))
    return combine_outputs(res.results)
